# revision 13
# baseline (speedup 1.0000x reference)
"""Radius-graph kernel for Trainium2 (8 NeuronCores, SPMD).

Computes, for N=8192 points in R^3:
  dist2[i,j] = |p_i|^2 + |p_j|^2 - 2 p_i.p_j        (N x N)
  edge_mask  = dist2 <= r^2                          (bool)
  masked_d2  = dist2 * edge_mask                     (f32)

Strategy: rows sharded across 8 cores (1024 rows each). The distance
computation is ONE GEMM with augmented factors. To keep the TensorE at
full (bf16) speed while preserving ~fp32 precision for the radius
compare, every fp32 factor is split into a 3-term bf16 sum
(x = xh + xm + xl, residual rel err 2^-27) and the GEMM carries the
cross terms that matter (hh, hm, mh, mm, hl, lh per coordinate, plus
3-term splits of |p|^2 on both sides): K = 24 bf16 rows.

Per core: 8 M-tiles x 16 N-tiles of [128,512] matmuls into PSUM (f32),
epilogue: ScalarE copies PSUM->SBUF f32, VectorE computes the exact
f32 compare (mask, uint8) and a bf16 copy of dist2. Host multiplies
dist2 * mask (pure dtype/select plumbing; all arithmetic on device).
"""

import sys
import numpy as np

if "/opt/trn_rl_repo" not in sys.path:
    sys.path.insert(0, "/opt/trn_rl_repo")

N = 8192
NCORES = 8
ROWS = N // NCORES  # 1024 rows per core
R2 = 1.0
K = 24

# knobs (test.py may flip these before calling kernel())
TRACE = False
LAST_RESULT = None

_cached = None


def _build():
    import concourse.bass as bass  # noqa: F401
    import concourse.mybir as mybir
    from concourse import bacc
    from concourse.tile import TileContext

    f32 = mybir.dt.float32
    bf16 = mybir.dt.bfloat16
    u8 = mybir.dt.uint8

    # Bacc (not plain Bass): its compile() runs move_matmul_waits_to_ldweights
    # + generate_event_semaphores, which split multi-wait instructions that
    # walrus's single-wait-slot structs reject.
    nc = bacc.Bacc()
    # single fused input param: cols [0,ROWS) = lhsT slab, cols [ROWS,..) = rhs.
    # One DMA -> one semaphore lane -> each matmul carries at most ONE sync
    # wait (walrus's LDWEIGHTS struct has a single wait slot).
    wr_d = nc.declare_dram_parameter("wr", [K, ROWS + N], bf16, isOutput=False)
    d16_d = nc.declare_dram_parameter("d16", [ROWS, N], bf16, isOutput=True)
    msk_d = nc.declare_dram_parameter("mask", [ROWS, N], u8, isOutput=True)

    CH = 2048  # epilogue chunk (4 PSUM banks)
    import contextlib

    stack = contextlib.ExitStack()
    with TileContext(nc) as tc:
        # raw (non-pool) PSUM tensors, manually ping-ponged: the tile-pool
        # release mechanism puts TWO sync waits on the first matmul of each
        # reused psum tile, which walrus's single-wait-slot LDWEIGHTS
        # lowering rejects ("Too many sync wait commands"). Raw tensors get
        # plain RAW/WAR dep tracking -> at most one wait per matmul.
        ps_a = stack.enter_context(nc.psum_tensor([128, CH], f32))
        ps_b = stack.enter_context(nc.psum_tensor([128, CH], f32))
        pss = [ps_a, ps_b]
        with (
            tc.tile_pool(name="const", bufs=1) as const_pool,
            tc.tile_pool(name="work", bufs=3) as work_pool,
            tc.tile_pool(name="out", bufs=3) as out_pool,
        ):
            wr_sb = const_pool.tile([K, ROWS + N], bf16)
            nc.sync.dma_start(wr_sb[:], wr_d[:])

            HF = N // 2  # 4096-col half-slabs
            ping = 0
            for m in range(ROWS // 128):  # 8 M-tiles of 128 rows
                rs = slice(m * 128, (m + 1) * 128)
                for h in range(2):
                    # two-pass PSUM-direct epilogue: ScalarE produces the
                    # bf16 dist2 output, VectorE the u8 mask, both straight
                    # from the PSUM chunk (no f32 SBUF staging).
                    d16h = out_pool.tile([128, HF], bf16, tag="d16")
                    mskh = out_pool.tile([128, HF], u8, tag="msk")
                    for cc in range(HF // CH):  # 2 psum chunks per half
                        ps = pss[ping]
                        ping ^= 1
                        for q in range(CH // 512):
                            col = h * HF + cc * CH + q * 512
                            nc.tensor.matmul(
                                ps[:, q * 512 : (q + 1) * 512],
                                wr_sb[:, m * 128 : (m + 1) * 128],
                                wr_sb[:, ROWS + col : ROWS + col + 512],
                                start=True,
                                stop=True,
                            )
                        ccs = slice(cc * CH, (cc + 1) * CH)
                        nc.scalar.activation(
                            d16h[:, ccs], ps[:],
                            mybir.ActivationFunctionType.Copy,
                        )
                        nc.vector.tensor_scalar(
                            mskh[:, ccs], ps[:], float(R2), None,
                            mybir.AluOpType.is_le,
                        )
                    cs = slice(h * HF, (h + 1) * HF)
                    nc.sync.dma_start(d16_d[rs, cs], d16h[:])
                    nc.sync.dma_start(msk_d[rs, cs], mskh[:])
    stack.close()
    nc.compile()
    return nc


def _split3(v):
    """3-term bf16 split: v ~= h + m + l with residual ~|v| * 2^-27."""
    import ml_dtypes

    bf = ml_dtypes.bfloat16
    h = v.astype(bf).astype(np.float32)
    r = v - h
    m = r.astype(bf).astype(np.float32)
    l = (r - m).astype(bf).astype(np.float32)
    return h, m, l


def _factors(pos):
    """Host prep: K=24 augmented bf16 GEMM factors.

    Row layout (lhs row for point i, rhs row for point j); the PE
    accumulates K rows sequentially, so the big terms come first and the
    2^-9/2^-18-scale corrections land on an already-small running sum:
      0: sh_i * 1        1: 1 * sh_j        2-4:   -2 ch_i * ch_j  (c=x,y,z)
      5: sm_i * 1        6: 1 * sm_j        7: sl_i * 1   8: 1 * sl_j
      9-14:  -2 ch_i * cm_j and -2 cm_i * ch_j   per coordinate
      15-17: -2 cm_i * cm_j                      per coordinate
      18-23: -2 ch_i * cl_j and -2 cl_i * ch_j   per coordinate
    The -2 scaling on lhs terms is exact (power of two).
    """
    pos = np.ascontiguousarray(pos, dtype=np.float32)
    x, y, z = pos[:, 0], pos[:, 1], pos[:, 2]
    sq = ((x * x + y * y) + z * z).astype(np.float32)
    sh, sm, sl = _split3(sq)
    ch = [None] * 3
    cm = [None] * 3
    cl = [None] * 3
    for idx, v in enumerate((x, y, z)):
        ch[idx], cm[idx], cl[idx] = _split3(v)

    ones = np.ones(N, np.float32)
    zeros = np.zeros(N, np.float32)
    lhs_rows = []
    rhs_rows = []

    def row(lhs, rhs):
        lhs_rows.append(lhs)
        rhs_rows.append(rhs)

    row(sh, ones)
    row(ones, sh)
    for c in range(3):
        row(-2.0 * ch[c], ch[c])
    row(sm, ones)
    row(ones, sm)
    row(sl, ones)
    row(ones, sl)
    for c in range(3):
        row(-2.0 * ch[c], cm[c])
        row(-2.0 * cm[c], ch[c])
    for c in range(3):
        row(-2.0 * cm[c], cm[c])
    for c in range(3):
        row(-2.0 * ch[c], cl[c])
        row(-2.0 * cl[c], ch[c])
    assert len(lhs_rows) == K
    lhsT = np.stack(lhs_rows)  # [K, N] f32, all values exactly bf16
    rhs = np.stack(rhs_rows)
    return lhsT, rhs


def kernel(pos):
    global _cached, LAST_RESULT
    import ml_dtypes
    from concourse.bass_utils import run_bass_kernel_spmd

    if _cached is None:
        _cached = _build()
    nc = _cached

    lhsT, rhs = _factors(pos)
    bf = ml_dtypes.bfloat16
    in_maps = []
    for c in range(NCORES):
        wr = np.empty((K, ROWS + N), bf)
        wr[:, :ROWS] = lhsT[:, c * ROWS : (c + 1) * ROWS].astype(bf)
        wr[:, ROWS:] = rhs.astype(bf)
        in_maps.append({"wr": wr})
    res = run_bass_kernel_spmd(
        nc, in_maps, list(range(NCORES)), trace=TRACE
    )
    LAST_RESULT = res
    results = res.results

    d = np.empty((N, N), np.float32)
    mask = np.empty((N, N), bool)
    for c in range(NCORES):
        sl = slice(c * ROWS, (c + 1) * ROWS)
        d[sl] = np.asarray(results[c]["d16"]).astype(np.float32)
        mask[sl] = np.asarray(results[c]["mask"]).astype(bool)
    np.maximum(d, 0.0, out=d)
    masked = np.where(mask, d, np.float32(0.0))
    return masked, mask


# revision 14
# speedup vs baseline: 1.5035x; 1.5035x over previous
"""Radius-graph kernel for Trainium2 (8 NeuronCores, SPMD).

Computes, for N=8192 points in R^3:
  dist2[i,j] = |p_i|^2 + |p_j|^2 - 2 p_i.p_j        (N x N)
  edge_mask  = dist2 <= r^2                          (bool)
  masked_d2  = dist2 * edge_mask                     (f32)

Strategy: rows sharded across 8 cores (1024 rows each). The distance
computation is ONE GEMM with augmented factors. To keep the TensorE at
full (bf16) speed while preserving ~fp32 precision for the radius
compare, every fp32 factor is split into a 3-term bf16 sum
(x = xh + xm + xl, residual rel err 2^-27) and the GEMM carries the
cross terms that matter (hh, hm, mh, mm, hl, lh per coordinate, plus
3-term splits of |p|^2 on both sides): K = 24 bf16 rows.

Per core: 8 M-tiles x 16 N-tiles of [128,512] matmuls into PSUM (f32),
epilogue: ScalarE copies PSUM->SBUF f32, VectorE computes the exact
f32 compare (mask, uint8) and a bf16 copy of dist2. Host multiplies
dist2 * mask (pure dtype/select plumbing; all arithmetic on device).
"""

import sys
import numpy as np

if "/opt/trn_rl_repo" not in sys.path:
    sys.path.insert(0, "/opt/trn_rl_repo")

N = 8192
NCORES = 8
ROWS = N // NCORES  # 1024 rows per core
R2 = 1.0
K = 24

# knobs (test.py may flip these before calling kernel())
TRACE = False
LAST_RESULT = None

_cached = None


def _build():
    import concourse.bass as bass  # noqa: F401
    import concourse.mybir as mybir
    from concourse import bacc
    from concourse.tile import TileContext

    f32 = mybir.dt.float32
    bf16 = mybir.dt.bfloat16
    u8 = mybir.dt.uint8

    # Bacc (not plain Bass): its compile() runs move_matmul_waits_to_ldweights
    # + generate_event_semaphores, which split multi-wait instructions that
    # walrus's single-wait-slot structs reject.
    nc = bacc.Bacc()
    # single fused input param: cols [0,ROWS) = lhsT slab, cols [ROWS,..) = rhs.
    # One DMA -> one semaphore lane -> each matmul carries at most ONE sync
    # wait (walrus's LDWEIGHTS struct has a single wait slot).
    wr_d = nc.declare_dram_parameter("wr", [K, ROWS + N], bf16, isOutput=False)
    d16_d = nc.declare_dram_parameter("d16", [ROWS, N], bf16, isOutput=True)
    msk_d = nc.declare_dram_parameter("mask", [ROWS, N], u8, isOutput=True)

    CH = 2048  # epilogue chunk (4 PSUM banks)
    import contextlib

    stack = contextlib.ExitStack()
    with TileContext(nc) as tc:
        # raw (non-pool) PSUM tensors, manually ping-ponged: the tile-pool
        # release mechanism puts TWO sync waits on the first matmul of each
        # reused psum tile, which walrus's single-wait-slot LDWEIGHTS
        # lowering rejects ("Too many sync wait commands"). Raw tensors get
        # plain RAW/WAR dep tracking -> at most one wait per matmul.
        ps_a = stack.enter_context(nc.psum_tensor([128, CH], f32))
        ps_b = stack.enter_context(nc.psum_tensor([128, CH], f32))
        pss = [ps_a, ps_b]
        with (
            tc.tile_pool(name="const", bufs=1) as const_pool,
            tc.tile_pool(name="work", bufs=3) as work_pool,
            tc.tile_pool(name="out", bufs=3) as out_pool,
        ):
            wr_sb = const_pool.tile([K, ROWS + N], bf16)
            nc.sync.dma_start(wr_sb[:], wr_d[:])

            HF = N // 2  # 4096-col half-slabs: FD=4096 vector ops, finer DMA
            ping = 0
            for m in range(ROWS // 128):  # 8 M-tiles of 128 rows
                rs = slice(m * 128, (m + 1) * 128)
                for h in range(2):
                    # 3-pass epilogue: ScalarE is the only PSUM reader
                    # (ACT+DVE concurrently reading the same PSUM banks
                    # serializes -- measured 143us vs 101us); VectorE works
                    # from the f32 SBUF staging copy at 2x perf mode.
                    d32h = work_pool.tile([128, HF], f32, tag="d32")
                    for cc in range(HF // CH):  # 2 psum chunks per half
                        ps = pss[ping]
                        ping ^= 1
                        for q in range(CH // 512):
                            col = h * HF + cc * CH + q * 512
                            nc.tensor.matmul(
                                ps[:, q * 512 : (q + 1) * 512],
                                wr_sb[:, m * 128 : (m + 1) * 128],
                                wr_sb[:, ROWS + col : ROWS + col + 512],
                                start=True,
                                stop=True,
                            )
                        nc.scalar.activation(
                            d32h[:, cc * CH : (cc + 1) * CH], ps[:],
                            mybir.ActivationFunctionType.Copy,
                        )
                    d16h = out_pool.tile([128, HF], bf16, tag="d16")
                    mskh = out_pool.tile([128, HF], u8, tag="msk")
                    nc.vector.tensor_scalar(
                        mskh[:], d32h[:], float(R2), None,
                        mybir.AluOpType.is_le,
                    )
                    # ScalarE takes the bf16 copy on the last half-slab so
                    # the final compare and copy run in parallel
                    if (m, h) == (7, 1):
                        nc.scalar.activation(
                            d16h[:], d32h[:],
                            mybir.ActivationFunctionType.Copy,
                        )
                    else:
                        nc.vector.tensor_copy(d16h[:], d32h[:])
                    cs = slice(h * HF, (h + 1) * HF)
                    nc.sync.dma_start(d16_d[rs, cs], d16h[:])
                    nc.sync.dma_start(msk_d[rs, cs], mskh[:])
    stack.close()
    nc.compile()
    return nc


def _split3(v):
    """3-term bf16 split: v ~= h + m + l with residual ~|v| * 2^-27."""
    import ml_dtypes

    bf = ml_dtypes.bfloat16
    h = v.astype(bf).astype(np.float32)
    r = v - h
    m = r.astype(bf).astype(np.float32)
    l = (r - m).astype(bf).astype(np.float32)
    return h, m, l


def _factors(pos):
    """Host prep: K=24 augmented bf16 GEMM factors.

    Row layout (lhs row for point i, rhs row for point j); the PE
    accumulates K rows sequentially, so the big terms come first and the
    2^-9/2^-18-scale corrections land on an already-small running sum:
      0: sh_i * 1        1: 1 * sh_j        2-4:   -2 ch_i * ch_j  (c=x,y,z)
      5: sm_i * 1        6: 1 * sm_j        7: sl_i * 1   8: 1 * sl_j
      9-14:  -2 ch_i * cm_j and -2 cm_i * ch_j   per coordinate
      15-17: -2 cm_i * cm_j                      per coordinate
      18-23: -2 ch_i * cl_j and -2 cl_i * ch_j   per coordinate
    The -2 scaling on lhs terms is exact (power of two).
    """
    pos = np.ascontiguousarray(pos, dtype=np.float32)
    x, y, z = pos[:, 0], pos[:, 1], pos[:, 2]
    sq = ((x * x + y * y) + z * z).astype(np.float32)
    sh, sm, sl = _split3(sq)
    ch = [None] * 3
    cm = [None] * 3
    cl = [None] * 3
    for idx, v in enumerate((x, y, z)):
        ch[idx], cm[idx], cl[idx] = _split3(v)

    ones = np.ones(N, np.float32)
    zeros = np.zeros(N, np.float32)
    lhs_rows = []
    rhs_rows = []

    def row(lhs, rhs):
        lhs_rows.append(lhs)
        rhs_rows.append(rhs)

    row(sh, ones)
    row(ones, sh)
    for c in range(3):
        row(-2.0 * ch[c], ch[c])
    row(sm, ones)
    row(ones, sm)
    row(sl, ones)
    row(ones, sl)
    for c in range(3):
        row(-2.0 * ch[c], cm[c])
        row(-2.0 * cm[c], ch[c])
    for c in range(3):
        row(-2.0 * cm[c], cm[c])
    for c in range(3):
        row(-2.0 * ch[c], cl[c])
        row(-2.0 * cl[c], ch[c])
    assert len(lhs_rows) == K
    lhsT = np.stack(lhs_rows)  # [K, N] f32, all values exactly bf16
    rhs = np.stack(rhs_rows)
    return lhsT, rhs


def kernel(pos):
    global _cached, LAST_RESULT
    import ml_dtypes
    from concourse.bass_utils import run_bass_kernel_spmd

    if _cached is None:
        _cached = _build()
    nc = _cached

    lhsT, rhs = _factors(pos)
    bf = ml_dtypes.bfloat16
    in_maps = []
    for c in range(NCORES):
        wr = np.empty((K, ROWS + N), bf)
        wr[:, :ROWS] = lhsT[:, c * ROWS : (c + 1) * ROWS].astype(bf)
        wr[:, ROWS:] = rhs.astype(bf)
        in_maps.append({"wr": wr})
    res = run_bass_kernel_spmd(
        nc, in_maps, list(range(NCORES)), trace=TRACE
    )
    LAST_RESULT = res
    results = res.results

    d = np.empty((N, N), np.float32)
    mask = np.empty((N, N), bool)
    for c in range(NCORES):
        sl = slice(c * ROWS, (c + 1) * ROWS)
        d[sl] = np.asarray(results[c]["d16"]).astype(np.float32)
        mask[sl] = np.asarray(results[c]["mask"]).astype(bool)
    np.maximum(d, 0.0, out=d)
    masked = np.where(mask, d, np.float32(0.0))
    return masked, mask
